# revision 6
# baseline (speedup 1.0000x reference)
"""Trainium2 Bass kernel for MultiHeadedAttentionBlur.

Math: qkv = x @ W_in^T (chunks v,k,q); per-head logits = SCALE * q @ k^T;
logits' key axis viewed as a 32x32 grid gets a 5x5 reflect-padded gaussian
blur; softmax over keys.

Key identity used here: the blur is linear over the key axis, so
blur(q @ k^T) = q @ (M @ k)^T with M = Bm (x) Bm the 1024x1024 blur matrix
(kron of two 32x32 1-D reflect-blur matrices). Blurring k (S x 64 per head)
instead of logits (S x S per head) removes ~16x of the blur FLOPs, and the
M-matmul doubles as the transpose that the logits matmul needs.

Sharding: data-parallel over batch (B=8 -> 8 cores). Each core computes its
batch element's 12 heads: [12, 1024, 1024] slice of the full [96, 1024, 1024]
output.

Per-core device pipeline (all matmuls in float32r = full-rate fp32):
  1. k  = x @ Wk^T            -> [s, f] layout (s on partitions)
  2. qT = Wq @ x^T            -> [f, s] layout (f on partitions)
  3. kbT = (M @ k)^T via matmul(lhsT=k, rhs=M^T) -> [f, s] layout
  4. per head h, per 128-query chunk: logits = qT_h^T @ kbT_h (K=64),
     exp via ScalarE (scale=0.125 folded in, accum_out = row sums),
     normalize via VectorE reciprocal + tensor_scalar_mul, DMA out.
"""

import numpy as np

S = 1024
E = 768
H = 12
D = 64
B = 8
NE = E // 128  # 6 e-tiles (contraction tiles of the projection)
NS = S // 128  # 8 s-tiles
NF = E // 128  # 6 f-tiles (output feature tiles; 2 heads per tile)
SCALE = 0.125
N_CORES = 8
KSIZE = 5
SIGMA = 1.0
GRID = 32


def _blur_matrix_1d():
    # Matches reference gaussian_kernel1d (fp32) + 'reflect' padding.
    x = (np.arange(KSIZE, dtype=np.float32) - (KSIZE - 1) / 2.0).astype(np.float32)
    g = np.exp(-0.5 * (x / SIGMA) ** 2).astype(np.float32)
    g = (g / g.sum()).astype(np.float32)
    pad = KSIZE // 2
    Bm = np.zeros((GRID, GRID), dtype=np.float32)
    for i in range(GRID):
        for t in range(-pad, pad + 1):
            j = i + t
            if j < 0:
                j = -j
            elif j > GRID - 1:
                j = 2 * (GRID - 1) - j
            Bm[i, j] += g[t + pad]
    return Bm


def _build():
    import concourse.bacc as bacc
    import concourse.mybir as mybir
    import concourse.tile as tile

    f32 = mybir.dt.float32
    f32r = mybir.dt.float32r
    AF = mybir.ActivationFunctionType

    nc = bacc.Bacc("TRN2", target_bir_lowering=False, debug=False)

    # float32r end-to-end: same bits as fp32 (PE rounds internally), but it
    # satisfies the BIR verifier's "rounded to FP32r" producer rule and runs
    # the PE at full rate (1 cyc/row at N>=256) instead of fp32's 4 cyc/row.
    xT = nc.dram_tensor("xT", [E, S], f32r, kind="ExternalInput")    # x[b].T
    wkT = nc.dram_tensor("wkT", [E, E], f32r, kind="ExternalInput")  # W_k.T
    wqT = nc.dram_tensor("wqT", [E, E], f32r, kind="ExternalInput")  # W_q.T
    mT = nc.dram_tensor("mT", [S, S], f32r, kind="ExternalInput")    # blur M.T
    out = nc.dram_tensor("out", [H, S, S], f32, kind="ExternalOutput")

    with tile.TileContext(nc) as tc:
        with (
            tc.tile_pool(name="persist", bufs=1) as pp,
            tc.tile_pool(name="qkb", bufs=2) as qp,
            tc.tile_pool(name="work", bufs=6) as wp,
            tc.tile_pool(name="stat", bufs=12) as sp,
            tc.tile_pool(name="pa", bufs=3, space="PSUM") as pa,
            tc.tile_pool(name="plg", bufs=2, space="PSUM") as plg,
        ):
            # ---- stage inputs in SBUF
            xts = []
            for i in range(NE):
                t = pp.tile([128, S], f32r, tag=f"x{i}", name=f"x{i}")
                nc.sync.dma_start(t[:], xT[i * 128:(i + 1) * 128, :])
                xts.append(t)
            wkts = []
            for i in range(NE):
                t = pp.tile([128, E], f32r, tag=f"wk{i}", name=f"wk{i}")
                nc.sync.dma_start(t[:], wkT[i * 128:(i + 1) * 128, :])
                wkts.append(t)
            wqts = []
            for i in range(NE):
                t = pp.tile([128, E], f32r, tag=f"wq{i}", name=f"wq{i}")
                nc.sync.dma_start(t[:], wqT[i * 128:(i + 1) * 128, :])
                wqts.append(t)
            mts = []
            for i in range(NS):
                t = pp.tile([128, S], f32r, tag=f"m{i}", name=f"m{i}")
                nc.sync.dma_start(t[:], mT[i * 128:(i + 1) * 128, :])
                mts.append(t)

            # ---- k = x @ Wk^T, laid out [s-tile partitions, f free].
            # f-major (256-wide f blocks) so the first head pair's kbT can
            # start after 1/3 of the k matmuls instead of all of them.
            kts = []
            for st in range(NS):
                t = pp.tile([128, E], f32r, tag=f"k{st}", name=f"k{st}")
                kts.append(t)
            for fb in range(3):
                f0 = fb * 256
                for st in range(NS):
                    ps = pa.tile([128, 512], f32, tag="pa", name=f"psk{st}_{fb}")
                    for et in range(NE):
                        nc.tensor.matmul(
                            ps[:, 0:256],
                            xts[et][:, st * 128:(st + 1) * 128],
                            wkts[et][:, f0:f0 + 256],
                            start=(et == 0),
                            stop=(et == NE - 1),
                        )
                    nc.vector.tensor_copy(kts[st][:, f0:f0 + 256], ps[:, 0:256])

            # ---- per f-tile: qT, kbT, then 2 heads of logits/softmax/out
            for ft in range(NF):
                qt = qp.tile([128, S], f32r, tag="qT", name=f"qT{ft}")
                for sb in range(2):
                    ps = pa.tile([128, 512], f32, tag="pa", name=f"psq{ft}_{sb}")
                    for et in range(NE):
                        nc.tensor.matmul(
                            ps[:],
                            wqts[et][:, ft * 128:(ft + 1) * 128],
                            xts[et][:, sb * 512:(sb + 1) * 512],
                            start=(et == 0),
                            stop=(et == NE - 1),
                        )
                    nc.vector.tensor_copy(qt[:, sb * 512:(sb + 1) * 512], ps[:])

                kbt = qp.tile([128, S], f32r, tag="kbT", name=f"kbT{ft}")
                for sb in range(2):
                    ps = pa.tile([128, 512], f32, tag="pa", name=f"psb{ft}_{sb}")
                    for st in range(NS):
                        nc.tensor.matmul(
                            ps[:],
                            kts[st][:, ft * 128:(ft + 1) * 128],
                            mts[st][:, sb * 512:(sb + 1) * 512],
                            start=(st == 0),
                            stop=(st == NS - 1),
                        )
                    nc.vector.tensor_copy(kbt[:, sb * 512:(sb + 1) * 512], ps[:])

                # interleave the two heads of this f-tile: their logits
                # matmuls live in different PE row groups (d-offset 0 vs 64),
                # so adjacent instructions can overlap on the array
                for qc in range(NS):
                    for hh in range(2):
                        h = 2 * ft + hh
                        off = hh * D
                        lg = plg.tile([128, S], f32, tag="lg", name=f"lg{h}_{qc}")
                        for kb in range(2):
                            nc.tensor.matmul(
                                lg[:, kb * 512:(kb + 1) * 512],
                                qt[off:off + D, qc * 128:(qc + 1) * 128],
                                kbt[off:off + D, kb * 512:(kb + 1) * 512],
                                start=True,
                                stop=True,
                            )
                        ex = wp.tile([128, S], f32, tag="exp", name=f"ex{h}_{qc}")
                        acc = sp.tile([128, 1], f32, tag="acc", name=f"ac{h}_{qc}")
                        nc.scalar.activation(
                            ex[:], lg[:], AF.Exp, scale=SCALE, accum_out=acc[:]
                        )
                        rs = sp.tile([128, 1], f32, tag="rs", name=f"rs{h}_{qc}")
                        nc.vector.reciprocal(rs[:], acc[:])
                        nc.vector.tensor_scalar_mul(ex[:], ex[:], rs[:])
                        nc.sync.dma_start(out[h, qc * 128:(qc + 1) * 128, :], ex[:])

    nc.compile()
    return nc


_CACHE = {}


def _get_nc():
    if "nc" not in _CACHE:
        _CACHE["nc"] = _build()
    return _CACHE["nc"]


def _make_in_maps(x, W_in):
    x = np.ascontiguousarray(np.asarray(x), dtype=np.float32)
    W_in = np.ascontiguousarray(np.asarray(W_in), dtype=np.float32)
    Bm = _blur_matrix_1d()
    M = np.kron(Bm, Bm).astype(np.float32)        # [s_out, s_in]
    mTn = np.ascontiguousarray(M.T)               # [s_in, s_out]
    wkTn = np.ascontiguousarray(W_in[E:2 * E, :].T)       # [E, E]
    wqTn = np.ascontiguousarray(W_in[2 * E:3 * E, :].T)   # [E, E]
    in_maps = []
    for b in range(N_CORES):
        in_maps.append(
            {
                "xT": np.ascontiguousarray(x[b].T),
                "wkT": wkTn,
                "wqT": wqTn,
                "mT": mTn,
            }
        )
    return in_maps


def _run(x, W_in, trace=False):
    from concourse.bass_utils import run_bass_kernel_spmd

    nc = _get_nc()
    in_maps = _make_in_maps(x, W_in)
    res = run_bass_kernel_spmd(nc, in_maps, list(range(N_CORES)), trace=trace)
    outs = [np.asarray(res.results[c]["out"]) for c in range(N_CORES)]
    full = np.concatenate(outs, axis=0)  # [B*H, S, S]
    return full, res


def kernel(x, W_in):
    full, _ = _run(x, W_in, trace=False)
    return full


# revision 11
# speedup vs baseline: 1.1320x; 1.1320x over previous
"""Trainium2 Bass kernel for MultiHeadedAttentionBlur.

Math: qkv = x @ W_in^T (chunks v,k,q); per-head logits = SCALE * q @ k^T;
logits' key axis viewed as a 32x32 grid gets a 5x5 reflect-padded gaussian
blur; softmax over keys.

Key identity used here: the blur is linear over the key axis, so
blur(q @ k^T) = q @ (M @ k)^T with M = Bm (x) Bm the 1024x1024 blur matrix
(kron of two 32x32 1-D reflect-blur matrices). Blurring k (S x 64 per head)
instead of logits (S x S per head) removes ~16x of the blur FLOPs, and the
M-matmul doubles as the transpose that the logits matmul needs.

Sharding: data-parallel over batch (B=8 -> 8 cores). Each core computes its
batch element's 12 heads: [12, 1024, 1024] slice of the full [96, 1024, 1024]
output.

Per-core device pipeline (all matmuls in float32r = full-rate fp32):
  1. k  = x @ Wk^T            -> [s, f] layout (s on partitions)
  2. qT = Wq @ x^T            -> [f, s] layout (f on partitions)
  3. kbT = (M @ k)^T via matmul(lhsT=k, rhs=M^T) -> [f, s] layout
  4. per head h, per 128-query chunk: logits = qT_h^T @ kbT_h (K=64),
     exp via ScalarE (scale=0.125 folded in, accum_out = row sums),
     normalize via VectorE reciprocal + tensor_scalar_mul, DMA out.
"""

import numpy as np

S = 1024
E = 768
H = 12
D = 64
B = 8
NE = E // 128  # 6 e-tiles (contraction tiles of the projection)
NS = S // 128  # 8 s-tiles
NF = E // 128  # 6 f-tiles (output feature tiles; 2 heads per tile)
SCALE = 0.125
N_CORES = 8
KSIZE = 5
SIGMA = 1.0
GRID = 32


def _blur_matrix_1d():
    # Matches reference gaussian_kernel1d (fp32) + 'reflect' padding.
    x = (np.arange(KSIZE, dtype=np.float32) - (KSIZE - 1) / 2.0).astype(np.float32)
    g = np.exp(-0.5 * (x / SIGMA) ** 2).astype(np.float32)
    g = (g / g.sum()).astype(np.float32)
    pad = KSIZE // 2
    Bm = np.zeros((GRID, GRID), dtype=np.float32)
    for i in range(GRID):
        for t in range(-pad, pad + 1):
            j = i + t
            if j < 0:
                j = -j
            elif j > GRID - 1:
                j = 2 * (GRID - 1) - j
            Bm[i, j] += g[t + pad]
    return Bm


def _build():
    import concourse.bacc as bacc
    import concourse.mybir as mybir
    import concourse.tile as tile

    f32 = mybir.dt.float32
    f32r = mybir.dt.float32r
    AF = mybir.ActivationFunctionType

    nc = bacc.Bacc("TRN2", target_bir_lowering=False, debug=False)

    # float32r end-to-end: same bits as fp32 (PE rounds internally), but it
    # satisfies the BIR verifier's "rounded to FP32r" producer rule and runs
    # the PE at full rate (1 cyc/row at N>=256) instead of fp32's 4 cyc/row.
    xT = nc.dram_tensor("xT", [E, S], f32r, kind="ExternalInput")    # x[b].T
    wkT = nc.dram_tensor("wkT", [E, E], f32r, kind="ExternalInput")  # W_k.T
    wqT = nc.dram_tensor("wqT", [E, E], f32r, kind="ExternalInput")  # W_q.T
    # blur-matrix generators: ab[s', 0:32] = Bm.T[s'//32, :], ab[s', 32:64] =
    # Bm.T[s'%32, :]; M^T[s', 32r+c] = ab[s', r] * ab[s', 32+c] is built on
    # device, saving a 4 MiB DMA on the startup critical path.
    ab = nc.dram_tensor("ab", [S, 2 * GRID], f32, kind="ExternalInput")
    out = nc.dram_tensor("out", [H, S, S], f32, kind="ExternalOutput")

    with tile.TileContext(nc) as tc:
        with (
            tc.tile_pool(name="persist", bufs=1) as pp,
            tc.tile_pool(name="qkb", bufs=2) as qp,
            tc.tile_pool(name="work", bufs=10) as wp,
            tc.tile_pool(name="stat", bufs=12) as sp,
            tc.tile_pool(name="pa", bufs=2, space="PSUM") as pa,
            tc.tile_pool(name="plg", bufs=3, space="PSUM") as plg,
        ):
            # ---- stage inputs in SBUF
            xts = []
            for i in range(NE):
                t = pp.tile([128, S], f32r, tag=f"x{i}", name=f"x{i}")
                nc.sync.dma_start(t[:], xT[i * 128:(i + 1) * 128, :])
                xts.append(t)
            wkts = []
            for i in range(NE):
                t = pp.tile([128, E], f32r, tag=f"wk{i}", name=f"wk{i}")
                nc.sync.dma_start(t[:], wkT[i * 128:(i + 1) * 128, :])
                wkts.append(t)
            wqts = []
            for i in range(NE):
                t = pp.tile([128, E], f32r, tag=f"wq{i}", name=f"wq{i}")
                nc.sync.dma_start(t[:], wqT[i * 128:(i + 1) * 128, :])
                wqts.append(t)
            # ---- build the blur matrix M^T in SBUF from the tiny generators
            mts = []
            for i in range(NS):
                g = pp.tile([128, 2 * GRID], f32, tag=f"g{i}", name=f"g{i}")
                nc.sync.dma_start(g[:], ab[i * 128:(i + 1) * 128, :])
                t = pp.tile([128, S], f32r, tag=f"m{i}", name=f"m{i}")
                nc.vector.tensor_mul(
                    t[:].rearrange("p (r c) -> p r c", c=GRID),
                    g[:, 0:GRID].to_broadcast([128, GRID, GRID]),
                    g[:].rearrange("p (a c) -> p a c", a=2)[:, 1:2, :]
                        .to_broadcast([128, GRID, GRID]),
                )
                mts.append(t)

            # ---- k = x @ Wk^T, laid out [s-tile partitions, f free]
            kts = []
            for st in range(NS):
                t = pp.tile([128, E], f32r, tag=f"k{st}", name=f"k{st}")
                kts.append(t)
            ncopy = 0
            for st in range(NS):
                for fb, (f0, fn) in enumerate(((0, 512), (512, 256))):
                    ps = pa.tile([128, 512], f32, tag="pa", name=f"psk{st}_{fb}")
                    for et in range(NE):
                        nc.tensor.matmul(
                            ps[:, 0:fn],
                            xts[et][:, st * 128:(st + 1) * 128],
                            wkts[et][:, f0:f0 + fn],
                            start=(et == 0),
                            stop=(et == NE - 1),
                        )
                    if ncopy % 2 == 0:
                        nc.scalar.copy(kts[st][:, f0:f0 + fn], ps[:, 0:fn])
                    else:
                        nc.vector.tensor_copy(kts[st][:, f0:f0 + fn], ps[:, 0:fn])
                    ncopy += 1

            # ---- per f-tile: qT, kbT, then 2 heads of logits/softmax/out
            for ft in range(NF):
                qt = qp.tile([128, S], f32r, tag="qT", name=f"qT{ft}")
                for sb in range(2):
                    ps = pa.tile([128, 512], f32, tag="pa", name=f"psq{ft}_{sb}")
                    for et in range(NE):
                        nc.tensor.matmul(
                            ps[:],
                            wqts[et][:, ft * 128:(ft + 1) * 128],
                            xts[et][:, sb * 512:(sb + 1) * 512],
                            start=(et == 0),
                            stop=(et == NE - 1),
                        )
                    nc.vector.tensor_copy(qt[:, sb * 512:(sb + 1) * 512], ps[:])

                kbt = qp.tile([128, S], f32r, tag="kbT", name=f"kbT{ft}")
                for sb in range(2):
                    ps = pa.tile([128, 512], f32, tag="pa", name=f"psb{ft}_{sb}")
                    for st in range(NS):
                        nc.tensor.matmul(
                            ps[:],
                            kts[st][:, ft * 128:(ft + 1) * 128],
                            mts[st][:, sb * 512:(sb + 1) * 512],
                            start=(st == 0),
                            stop=(st == NS - 1),
                        )
                    nc.vector.tensor_copy(kbt[:, sb * 512:(sb + 1) * 512], ps[:])

                # interleave the two heads of this f-tile: their logits
                # matmuls live in different PE row groups (d-offset 0 vs 64),
                # so adjacent instructions can overlap on the array
                for qc in range(NS):
                    for hh in range(2):
                        h = 2 * ft + hh
                        off = hh * D
                        lg = plg.tile([128, S], f32, tag="lg", name=f"lg{h}_{qc}")
                        for kb in range(2):
                            nc.tensor.matmul(
                                lg[:, kb * 512:(kb + 1) * 512],
                                qt[off:off + D, qc * 128:(qc + 1) * 128],
                                kbt[off:off + D, kb * 512:(kb + 1) * 512],
                                start=True,
                                stop=True,
                            )
                        ex = wp.tile([128, S], f32, tag="exp", name=f"ex{h}_{qc}")
                        acc = sp.tile([128, 1], f32, tag="acc", name=f"ac{h}_{qc}")
                        nc.scalar.activation(
                            ex[:], lg[:], AF.Exp, scale=SCALE, accum_out=acc[:]
                        )
                        rs = sp.tile([128, 1], f32, tag="rs", name=f"rs{h}_{qc}")
                        nc.vector.reciprocal(rs[:], acc[:])
                        nc.vector.tensor_scalar_mul(ex[:], ex[:], rs[:])
                        nc.sync.dma_start(out[h, qc * 128:(qc + 1) * 128, :], ex[:])

    nc.compile()
    return nc


_CACHE = {}


def _get_nc():
    if "nc" not in _CACHE:
        _CACHE["nc"] = _build()
    return _CACHE["nc"]


def _make_in_maps(x, W_in):
    x = np.ascontiguousarray(np.asarray(x), dtype=np.float32)
    W_in = np.ascontiguousarray(np.asarray(W_in), dtype=np.float32)
    Bm = _blur_matrix_1d()
    # M^T[s', 32r+c] = Bm[r, s'//32] * Bm[c, s'%32]; ship the two 32-wide
    # generator tables and take the outer product on device.
    sp = np.arange(S)
    abn = np.concatenate([Bm.T[sp // GRID, :], Bm.T[sp % GRID, :]], axis=1)
    abn = np.ascontiguousarray(abn, dtype=np.float32)     # [S, 64]
    wkTn = np.ascontiguousarray(W_in[E:2 * E, :].T)       # [E, E]
    wqTn = np.ascontiguousarray(W_in[2 * E:3 * E, :].T)   # [E, E]
    in_maps = []
    for b in range(N_CORES):
        in_maps.append(
            {
                "xT": np.ascontiguousarray(x[b].T),
                "wkT": wkTn,
                "wqT": wqTn,
                "ab": abn,
            }
        )
    return in_maps


def _run(x, W_in, trace=False):
    from concourse.bass_utils import run_bass_kernel_spmd

    nc = _get_nc()
    in_maps = _make_in_maps(x, W_in)
    res = run_bass_kernel_spmd(nc, in_maps, list(range(N_CORES)), trace=trace)
    outs = [np.asarray(res.results[c]["out"]) for c in range(N_CORES)]
    full = np.concatenate(outs, axis=0)  # [B*H, S, S]
    return full, res


def kernel(x, W_in):
    full, _ = _run(x, W_in, trace=False)
    return full


# revision 13
# speedup vs baseline: 1.1758x; 1.0387x over previous
"""Trainium2 Bass kernel for MultiHeadedAttentionBlur.

Math: qkv = x @ W_in^T (chunks v,k,q); per-head logits = SCALE * q @ k^T;
logits' key axis viewed as a 32x32 grid gets a 5x5 reflect-padded gaussian
blur; softmax over keys.

Key identity used here: the blur is linear over the key axis, so
blur(q @ k^T) = q @ (M @ k)^T with M = Bm (x) Bm the 1024x1024 blur matrix
(kron of two 32x32 1-D reflect-blur matrices). Blurring k (S x 64 per head)
instead of logits (S x S per head) removes ~16x of the blur FLOPs, and the
M-matmul doubles as the transpose that the logits matmul needs.

Sharding: data-parallel over batch (B=8 -> 8 cores). Each core computes its
batch element's 12 heads: [12, 1024, 1024] slice of the full [96, 1024, 1024]
output.

Per-core device pipeline (all matmuls in float32r = full-rate fp32):
  1. k  = x @ Wk^T            -> [s, f] layout (s on partitions)
  2. qT = Wq @ x^T            -> [f, s] layout (f on partitions)
  3. kbT = (M @ k)^T via matmul(lhsT=k, rhs=M^T) -> [f, s] layout
  4. per head h, per 128-query chunk: logits = qT_h^T @ kbT_h (K=64),
     exp via ScalarE (scale=0.125 folded in, accum_out = row sums),
     normalize via VectorE reciprocal + tensor_scalar_mul, DMA out.
"""

import numpy as np

S = 1024
E = 768
H = 12
D = 64
B = 8
NE = E // 128  # 6 e-tiles (contraction tiles of the projection)
NS = S // 128  # 8 s-tiles
NF = E // 128  # 6 f-tiles (output feature tiles; 2 heads per tile)
SCALE = 0.125
N_CORES = 8
KSIZE = 5
SIGMA = 1.0
GRID = 32


def _blur_matrix_1d():
    # Matches reference gaussian_kernel1d (fp32) + 'reflect' padding.
    x = (np.arange(KSIZE, dtype=np.float32) - (KSIZE - 1) / 2.0).astype(np.float32)
    g = np.exp(-0.5 * (x / SIGMA) ** 2).astype(np.float32)
    g = (g / g.sum()).astype(np.float32)
    pad = KSIZE // 2
    Bm = np.zeros((GRID, GRID), dtype=np.float32)
    for i in range(GRID):
        for t in range(-pad, pad + 1):
            j = i + t
            if j < 0:
                j = -j
            elif j > GRID - 1:
                j = 2 * (GRID - 1) - j
            Bm[i, j] += g[t + pad]
    return Bm


def _build():
    import concourse.bacc as bacc
    import concourse.mybir as mybir
    import concourse.tile as tile

    f32 = mybir.dt.float32
    f32r = mybir.dt.float32r
    AF = mybir.ActivationFunctionType

    nc = bacc.Bacc("TRN2", target_bir_lowering=False, debug=False)

    # float32r end-to-end: same bits as fp32 (PE rounds internally), but it
    # satisfies the BIR verifier's "rounded to FP32r" producer rule and runs
    # the PE at full rate (1 cyc/row at N>=256) instead of fp32's 4 cyc/row.
    xT = nc.dram_tensor("xT", [E, S], f32r, kind="ExternalInput")    # x[b].T
    wkT = nc.dram_tensor("wkT", [E, E], f32r, kind="ExternalInput")  # W_k.T
    wqT = nc.dram_tensor("wqT", [E, E], f32r, kind="ExternalInput")  # W_q.T
    # blur-matrix generators: ab[s', 0:32] = Bm.T[s'//32, :], ab[s', 32:64] =
    # Bm.T[s'%32, :]; M^T[s', 32r+c] = ab[s', r] * ab[s', 32+c] is built on
    # device, saving a 4 MiB DMA on the startup critical path.
    ab = nc.dram_tensor("ab", [S, 2 * GRID], f32, kind="ExternalInput")
    out = nc.dram_tensor("out", [H, S, S], f32, kind="ExternalOutput")

    with tile.TileContext(nc) as tc:
        with (
            tc.tile_pool(name="persist", bufs=1) as pp,
            tc.tile_pool(name="qkb", bufs=2) as qp,
            tc.tile_pool(name="work", bufs=10) as wp,
            tc.tile_pool(name="stat", bufs=12) as sp,
            tc.tile_pool(name="pa", bufs=2, space="PSUM") as pa,
            tc.tile_pool(name="plg", bufs=3, space="PSUM") as plg,
        ):
            # ---- stage inputs in SBUF (ab first: tiny, unblocks the blur-
            # matrix build; x+wk next: the k-phase critical path; wq last)
            gts = []
            for i in range(NS):
                g = pp.tile([128, 2 * GRID], f32, tag=f"g{i}", name=f"g{i}")
                nc.sync.dma_start(g[:], ab[i * 128:(i + 1) * 128, :])
                gts.append(g)
            xts = []
            for i in range(NE):
                t = pp.tile([128, S], f32r, tag=f"x{i}", name=f"x{i}")
                nc.sync.dma_start(t[:], xT[i * 128:(i + 1) * 128, :])
                xts.append(t)
            wkts = []
            for i in range(NE):
                t = pp.tile([128, E], f32r, tag=f"wk{i}", name=f"wk{i}")
                nc.sync.dma_start(t[:], wkT[i * 128:(i + 1) * 128, :])
                wkts.append(t)
            wqts = []
            for i in range(NE):
                t = pp.tile([128, E], f32r, tag=f"wq{i}", name=f"wq{i}")
                nc.sync.dma_start(t[:], wqT[i * 128:(i + 1) * 128, :])
                wqts.append(t)

            # ---- build the blur matrix M^T in SBUF from the tiny generators
            mts = []
            for i in range(NS):
                g = gts[i]
                t = pp.tile([128, S], f32r, tag=f"m{i}", name=f"m{i}")
                nc.vector.tensor_mul(
                    t[:].rearrange("p (r c) -> p r c", c=GRID),
                    g[:, 0:GRID].to_broadcast([128, GRID, GRID]),
                    g[:].rearrange("p (a c) -> p a c", a=2)[:, 1:2, :]
                        .to_broadcast([128, GRID, GRID]),
                )
                mts.append(t)

            kts = []
            for st in range(NS):
                t = pp.tile([128, E], f32r, tag=f"k{st}", name=f"k{st}")
                kts.append(t)

            def k_block(f0, fn):
                # k[:, f0:f0+fn] = (x @ Wk^T)[:, f0:f0+fn] for all s-tiles
                for st in range(NS):
                    ps = pa.tile([128, 512], f32, tag="pa", name=f"psk{st}_{f0}")
                    for et in range(NE):
                        nc.tensor.matmul(
                            ps[:, 0:fn],
                            xts[et][:, st * 128:(st + 1) * 128],
                            wkts[et][:, f0:f0 + fn],
                            start=(et == 0),
                            stop=(et == NE - 1),
                        )
                    if st % 2 == 0:
                        nc.scalar.copy(kts[st][:, f0:f0 + fn], ps[:, 0:fn])
                    else:
                        nc.vector.tensor_copy(kts[st][:, f0:f0 + fn], ps[:, 0:fn])

            def q_proj(ft):
                qt = qp.tile([128, S], f32r, tag="qT", name=f"qT{ft}", bufs=5)
                for sb in range(2):
                    ps = pa.tile([128, 512], f32, tag="pa", name=f"psq{ft}_{sb}")
                    for et in range(NE):
                        nc.tensor.matmul(
                            ps[:],
                            wqts[et][:, ft * 128:(ft + 1) * 128],
                            xts[et][:, sb * 512:(sb + 1) * 512],
                            start=(et == 0),
                            stop=(et == NE - 1),
                        )
                    nc.vector.tensor_copy(qt[:, sb * 512:(sb + 1) * 512], ps[:])
                return qt

            def head_pair(ft, qt):
                # blur: kbT[ft] = ((Bm x Bm) @ k)^T rows [128ft, 128ft+128)
                kbt = qp.tile([128, S], f32r, tag="kbT", name=f"kbT{ft}")
                for sb in range(2):
                    ps = pa.tile([128, 512], f32, tag="pa", name=f"psb{ft}_{sb}")
                    for st in range(NS):
                        nc.tensor.matmul(
                            ps[:],
                            kts[st][:, ft * 128:(ft + 1) * 128],
                            mts[st][:, sb * 512:(sb + 1) * 512],
                            start=(st == 0),
                            stop=(st == NS - 1),
                        )
                    nc.vector.tensor_copy(kbt[:, sb * 512:(sb + 1) * 512], ps[:])

                # two heads interleaved: adjacent logits matmuls sit in
                # different PE row groups (d-offset 0 vs 64)
                for qc in range(NS):
                    for hh in range(2):
                        h = 2 * ft + hh
                        off = hh * D
                        lg = plg.tile([128, S], f32, tag="lg", name=f"lg{h}_{qc}")
                        for kb in range(2):
                            nc.tensor.matmul(
                                lg[:, kb * 512:(kb + 1) * 512],
                                qt[off:off + D, qc * 128:(qc + 1) * 128],
                                kbt[off:off + D, kb * 512:(kb + 1) * 512],
                                start=True,
                                stop=True,
                            )
                        ex = wp.tile([128, S], f32, tag="exp", name=f"ex{h}_{qc}")
                        acc = sp.tile([128, 1], f32, tag="acc", name=f"ac{h}_{qc}")
                        nc.scalar.activation(
                            ex[:], lg[:], AF.Exp, scale=SCALE, accum_out=acc[:]
                        )
                        rs = sp.tile([128, 1], f32, tag="rs", name=f"rs{h}_{qc}")
                        nc.vector.reciprocal(rs[:], acc[:])
                        nc.vector.tensor_scalar_mul(ex[:], ex[:], rs[:])
                        nc.sync.dma_start(out[h, qc * 128:(qc + 1) * 128, :], ex[:])

            # minimal chain to the first output: k cols 0:256 -> ft0 -> ft1;
            # then the rest of k and the remaining qT projections run while
            # the ft0/ft1 output backlog drains; then ft2..5
            k_block(0, 256)
            head_pair(0, q_proj(0))
            head_pair(1, q_proj(1))
            k_block(256, 512)
            qts = [q_proj(ft) for ft in range(2, NF)]
            for ft in range(2, NF):
                head_pair(ft, qts[ft - 2])

    nc.compile()
    return nc


_CACHE = {}


def _get_nc():
    if "nc" not in _CACHE:
        _CACHE["nc"] = _build()
    return _CACHE["nc"]


def _make_in_maps(x, W_in):
    x = np.ascontiguousarray(np.asarray(x), dtype=np.float32)
    W_in = np.ascontiguousarray(np.asarray(W_in), dtype=np.float32)
    Bm = _blur_matrix_1d()
    # M^T[s', 32r+c] = Bm[r, s'//32] * Bm[c, s'%32]; ship the two 32-wide
    # generator tables and take the outer product on device.
    sp = np.arange(S)
    abn = np.concatenate([Bm.T[sp // GRID, :], Bm.T[sp % GRID, :]], axis=1)
    abn = np.ascontiguousarray(abn, dtype=np.float32)     # [S, 64]
    wkTn = np.ascontiguousarray(W_in[E:2 * E, :].T)       # [E, E]
    wqTn = np.ascontiguousarray(W_in[2 * E:3 * E, :].T)   # [E, E]
    in_maps = []
    for b in range(N_CORES):
        in_maps.append(
            {
                "xT": np.ascontiguousarray(x[b].T),
                "wkT": wkTn,
                "wqT": wqTn,
                "ab": abn,
            }
        )
    return in_maps


def _run(x, W_in, trace=False):
    from concourse.bass_utils import run_bass_kernel_spmd

    nc = _get_nc()
    in_maps = _make_in_maps(x, W_in)
    res = run_bass_kernel_spmd(nc, in_maps, list(range(N_CORES)), trace=trace)
    outs = [np.asarray(res.results[c]["out"]) for c in range(N_CORES)]
    full = np.concatenate(outs, axis=0)  # [B*H, S, S]
    return full, res


def kernel(x, W_in):
    full, _ = _run(x, W_in, trace=False)
    return full


# revision 15
# speedup vs baseline: 1.1906x; 1.0125x over previous
"""Trainium2 Bass kernel for MultiHeadedAttentionBlur.

Math: qkv = x @ W_in^T (chunks v,k,q); per-head logits = SCALE * q @ k^T;
logits' key axis viewed as a 32x32 grid gets a 5x5 reflect-padded gaussian
blur; softmax over keys.

Key identity used here: the blur is linear over the key axis, so
blur(q @ k^T) = q @ (M @ k)^T with M = Bm (x) Bm the 1024x1024 blur matrix
(kron of two 32x32 1-D reflect-blur matrices). Blurring k (S x 64 per head)
instead of logits (S x S per head) removes ~16x of the blur FLOPs, and the
M-matmul doubles as the transpose that the logits matmul needs.

Sharding: data-parallel over batch (B=8 -> 8 cores). Each core computes its
batch element's 12 heads: [12, 1024, 1024] slice of the full [96, 1024, 1024]
output.

Per-core device pipeline (all matmuls in float32r = full-rate fp32):
  1. k  = x @ Wk^T            -> [s, f] layout (s on partitions)
  2. qT = Wq @ x^T            -> [f, s] layout (f on partitions)
  3. kbT = (M @ k)^T via matmul(lhsT=k, rhs=M^T) -> [f, s] layout
  4. per head h, per 128-query chunk: logits = qT_h^T @ kbT_h (K=64),
     exp via ScalarE (scale=0.125 folded in, accum_out = row sums),
     normalize via VectorE reciprocal + tensor_scalar_mul, DMA out.
"""

import numpy as np

S = 1024
E = 768
H = 12
D = 64
B = 8
NE = E // 128  # 6 e-tiles (contraction tiles of the projection)
NS = S // 128  # 8 s-tiles
NF = E // 128  # 6 f-tiles (output feature tiles; 2 heads per tile)
SCALE = 0.125
N_CORES = 8
KSIZE = 5
SIGMA = 1.0
GRID = 32


def _blur_matrix_1d():
    # Matches reference gaussian_kernel1d (fp32) + 'reflect' padding.
    x = (np.arange(KSIZE, dtype=np.float32) - (KSIZE - 1) / 2.0).astype(np.float32)
    g = np.exp(-0.5 * (x / SIGMA) ** 2).astype(np.float32)
    g = (g / g.sum()).astype(np.float32)
    pad = KSIZE // 2
    Bm = np.zeros((GRID, GRID), dtype=np.float32)
    for i in range(GRID):
        for t in range(-pad, pad + 1):
            j = i + t
            if j < 0:
                j = -j
            elif j > GRID - 1:
                j = 2 * (GRID - 1) - j
            Bm[i, j] += g[t + pad]
    return Bm


def _build():
    import concourse.bacc as bacc
    import concourse.mybir as mybir
    import concourse.tile as tile

    f32 = mybir.dt.float32
    f32r = mybir.dt.float32r
    AF = mybir.ActivationFunctionType

    nc = bacc.Bacc("TRN2", target_bir_lowering=False, debug=False)

    # float32r end-to-end: same bits as fp32 (PE rounds internally), but it
    # satisfies the BIR verifier's "rounded to FP32r" producer rule and runs
    # the PE at full rate (1 cyc/row at N>=256) instead of fp32's 4 cyc/row.
    xT = nc.dram_tensor("xT", [E, S], f32r, kind="ExternalInput")    # x[b].T
    wkT = nc.dram_tensor("wkT", [E, E], f32r, kind="ExternalInput")  # W_k.T
    wqT = nc.dram_tensor("wqT", [E, E], f32r, kind="ExternalInput")  # W_q.T
    # blur-matrix generators: ab[s', 0:32] = Bm.T[s'//32, :], ab[s', 32:64] =
    # Bm.T[s'%32, :]; M^T[s', 32r+c] = ab[s', r] * ab[s', 32+c] is built on
    # device, saving a 4 MiB DMA on the startup critical path.
    ab = nc.dram_tensor("ab", [S, 2 * GRID], f32, kind="ExternalInput")
    out = nc.dram_tensor("out", [H, S, S], f32, kind="ExternalOutput")

    with tile.TileContext(nc) as tc:
        with (
            tc.tile_pool(name="persist", bufs=1) as pp,
            tc.tile_pool(name="qkb", bufs=2) as qp,
            tc.tile_pool(name="work", bufs=10) as wp,
            tc.tile_pool(name="stat", bufs=12) as sp,
            tc.tile_pool(name="pa", bufs=2, space="PSUM") as pa,
            tc.tile_pool(name="plg", bufs=3, space="PSUM") as plg,
        ):
            # ---- stage inputs in SBUF (ab first: tiny, unblocks the blur-
            # matrix build; x+wk next: the k-phase critical path; wq last)
            gts = []
            for i in range(NS):
                g = pp.tile([128, 2 * GRID], f32, tag=f"g{i}", name=f"g{i}")
                nc.sync.dma_start(g[:], ab[i * 128:(i + 1) * 128, :])
                gts.append(g)
            xts = []
            for i in range(NE):
                t = pp.tile([128, S], f32r, tag=f"x{i}", name=f"x{i}")
                nc.sync.dma_start(t[:], xT[i * 128:(i + 1) * 128, :])
                xts.append(t)
            # wk cols 0:256 arrive first (all the first k-chunk needs); the
            # rest of wk comes after wq so the first outputs start sooner
            wkts = []
            for i in range(NE):
                t = pp.tile([128, E], f32r, tag=f"wk{i}", name=f"wk{i}")
                nc.sync.dma_start(t[:, 0:256], wkT[i * 128:(i + 1) * 128, 0:256])
                wkts.append(t)
            wqts = []
            for i in range(NE):
                t = pp.tile([128, E], f32r, tag=f"wq{i}", name=f"wq{i}")
                nc.sync.dma_start(t[:], wqT[i * 128:(i + 1) * 128, :])
                wqts.append(t)
            for i in range(NE):
                nc.sync.dma_start(
                    wkts[i][:, 256:E], wkT[i * 128:(i + 1) * 128, 256:E]
                )

            # ---- build the blur matrix M^T in SBUF from the tiny generators
            mts = []
            for i in range(NS):
                g = gts[i]
                t = pp.tile([128, S], f32r, tag=f"m{i}", name=f"m{i}")
                nc.vector.tensor_mul(
                    t[:].rearrange("p (r c) -> p r c", c=GRID),
                    g[:, 0:GRID].to_broadcast([128, GRID, GRID]),
                    g[:].rearrange("p (a c) -> p a c", a=2)[:, 1:2, :]
                        .to_broadcast([128, GRID, GRID]),
                )
                mts.append(t)

            kts = []
            for st in range(NS):
                t = pp.tile([128, E], f32r, tag=f"k{st}", name=f"k{st}")
                kts.append(t)

            def k_block(f0, fn):
                # k[:, f0:f0+fn] = (x @ Wk^T)[:, f0:f0+fn] for all s-tiles
                for st in range(NS):
                    ps = pa.tile([128, 512], f32, tag="pa", name=f"psk{st}_{f0}")
                    for et in range(NE):
                        nc.tensor.matmul(
                            ps[:, 0:fn],
                            xts[et][:, st * 128:(st + 1) * 128],
                            wkts[et][:, f0:f0 + fn],
                            start=(et == 0),
                            stop=(et == NE - 1),
                        )
                    if st % 2 == 0:
                        nc.scalar.copy(kts[st][:, f0:f0 + fn], ps[:, 0:fn])
                    else:
                        nc.vector.tensor_copy(kts[st][:, f0:f0 + fn], ps[:, 0:fn])

            def q_proj(ft):
                qt = qp.tile([128, S], f32r, tag="qT", name=f"qT{ft}", bufs=5)
                for sb in range(2):
                    ps = pa.tile([128, 512], f32, tag="pa", name=f"psq{ft}_{sb}")
                    for et in range(NE):
                        nc.tensor.matmul(
                            ps[:],
                            wqts[et][:, ft * 128:(ft + 1) * 128],
                            xts[et][:, sb * 512:(sb + 1) * 512],
                            start=(et == 0),
                            stop=(et == NE - 1),
                        )
                    nc.vector.tensor_copy(qt[:, sb * 512:(sb + 1) * 512], ps[:])
                return qt

            def head_pair(ft, qt):
                # blur: kbT[ft] = ((Bm x Bm) @ k)^T rows [128ft, 128ft+128)
                kbt = qp.tile([128, S], f32r, tag="kbT", name=f"kbT{ft}")
                for sb in range(2):
                    ps = pa.tile([128, 512], f32, tag="pa", name=f"psb{ft}_{sb}")
                    for st in range(NS):
                        nc.tensor.matmul(
                            ps[:],
                            kts[st][:, ft * 128:(ft + 1) * 128],
                            mts[st][:, sb * 512:(sb + 1) * 512],
                            start=(st == 0),
                            stop=(st == NS - 1),
                        )
                    nc.vector.tensor_copy(kbt[:, sb * 512:(sb + 1) * 512], ps[:])

                # two heads interleaved: adjacent logits matmuls sit in
                # different PE row groups (d-offset 0 vs 64)
                for qc in range(NS):
                    for hh in range(2):
                        h = 2 * ft + hh
                        off = hh * D
                        lg = plg.tile([128, S], f32, tag="lg", name=f"lg{h}_{qc}")
                        for kb in range(2):
                            nc.tensor.matmul(
                                lg[:, kb * 512:(kb + 1) * 512],
                                qt[off:off + D, qc * 128:(qc + 1) * 128],
                                kbt[off:off + D, kb * 512:(kb + 1) * 512],
                                start=True,
                                stop=True,
                            )
                        ex = wp.tile([128, S], f32, tag="exp", name=f"ex{h}_{qc}")
                        acc = sp.tile([128, 1], f32, tag="acc", name=f"ac{h}_{qc}")
                        nc.scalar.activation(
                            ex[:], lg[:], AF.Exp, scale=SCALE, accum_out=acc[:]
                        )
                        rs = sp.tile([128, 1], f32, tag="rs", name=f"rs{h}_{qc}")
                        nc.vector.reciprocal(rs[:], acc[:])
                        nc.vector.tensor_scalar_mul(ex[:], ex[:], rs[:])
                        nc.sync.dma_start(out[h, qc * 128:(qc + 1) * 128, :], ex[:])

            # minimal chain to the first output: k cols 0:256 -> ft0 -> ft1.
            # The remaining k columns and qT projections are spread between
            # head pairs so the PE never outruns the output-DMA backlog.
            k_block(0, 256)
            head_pair(0, q_proj(0))
            head_pair(1, q_proj(1))
            k_block(256, 256)
            qt2 = q_proj(2)
            qt3 = q_proj(3)
            head_pair(2, qt2)
            k_block(512, 256)
            qt4 = q_proj(4)
            qt5 = q_proj(5)
            head_pair(3, qt3)
            head_pair(4, qt4)
            head_pair(5, qt5)

    nc.compile()
    return nc


_CACHE = {}


def _get_nc():
    if "nc" not in _CACHE:
        _CACHE["nc"] = _build()
    return _CACHE["nc"]


def _make_in_maps(x, W_in):
    x = np.ascontiguousarray(np.asarray(x), dtype=np.float32)
    W_in = np.ascontiguousarray(np.asarray(W_in), dtype=np.float32)
    Bm = _blur_matrix_1d()
    # M^T[s', 32r+c] = Bm[r, s'//32] * Bm[c, s'%32]; ship the two 32-wide
    # generator tables and take the outer product on device.
    sp = np.arange(S)
    abn = np.concatenate([Bm.T[sp // GRID, :], Bm.T[sp % GRID, :]], axis=1)
    abn = np.ascontiguousarray(abn, dtype=np.float32)     # [S, 64]
    wkTn = np.ascontiguousarray(W_in[E:2 * E, :].T)       # [E, E]
    wqTn = np.ascontiguousarray(W_in[2 * E:3 * E, :].T)   # [E, E]
    in_maps = []
    for b in range(N_CORES):
        in_maps.append(
            {
                "xT": np.ascontiguousarray(x[b].T),
                "wkT": wkTn,
                "wqT": wqTn,
                "ab": abn,
            }
        )
    return in_maps


def _run(x, W_in, trace=False):
    from concourse.bass_utils import run_bass_kernel_spmd

    nc = _get_nc()
    in_maps = _make_in_maps(x, W_in)
    res = run_bass_kernel_spmd(nc, in_maps, list(range(N_CORES)), trace=trace)
    outs = [np.asarray(res.results[c]["out"]) for c in range(N_CORES)]
    full = np.concatenate(outs, axis=0)  # [B*H, S, S]
    return full, res


def kernel(x, W_in):
    full, _ = _run(x, W_in, trace=False)
    return full


# revision 18
# speedup vs baseline: 1.1967x; 1.0051x over previous
"""Trainium2 Bass kernel for MultiHeadedAttentionBlur.

Math: qkv = x @ W_in^T (chunks v,k,q); per-head logits = SCALE * q @ k^T;
logits' key axis viewed as a 32x32 grid gets a 5x5 reflect-padded gaussian
blur; softmax over keys.

Key identity used here: the blur is linear over the key axis, so
blur(q @ k^T) = q @ (M @ k)^T with M = Bm (x) Bm the 1024x1024 blur matrix
(kron of two 32x32 1-D reflect-blur matrices). Blurring k (S x 64 per head)
instead of logits (S x S per head) removes ~16x of the blur FLOPs, and the
M-matmul doubles as the transpose that the logits matmul needs.

Sharding: data-parallel over batch (B=8 -> 8 cores). Each core computes its
batch element's 12 heads: [12, 1024, 1024] slice of the full [96, 1024, 1024]
output.

Per-core device pipeline (all matmuls in float32r = full-rate fp32):
  1. k  = x @ Wk^T            -> [s, f] layout (s on partitions)
  2. qT = Wq @ x^T            -> [f, s] layout (f on partitions)
  3. kbT = (M @ k)^T via matmul(lhsT=k, rhs=M^T) -> [f, s] layout
  4. per head h, per 128-query chunk: logits = qT_h^T @ kbT_h (K=64),
     exp via ScalarE (scale=0.125 folded in, accum_out = row sums),
     normalize via VectorE reciprocal + tensor_scalar_mul, DMA out.
"""

import numpy as np

S = 1024
E = 768
H = 12
D = 64
B = 8
NE = E // 128  # 6 e-tiles (contraction tiles of the projection)
NS = S // 128  # 8 s-tiles
NF = E // 128  # 6 f-tiles (output feature tiles; 2 heads per tile)
SCALE = 0.125
N_CORES = 8
KSIZE = 5
SIGMA = 1.0
GRID = 32


def _blur_matrix_1d():
    # Matches reference gaussian_kernel1d (fp32) + 'reflect' padding.
    x = (np.arange(KSIZE, dtype=np.float32) - (KSIZE - 1) / 2.0).astype(np.float32)
    g = np.exp(-0.5 * (x / SIGMA) ** 2).astype(np.float32)
    g = (g / g.sum()).astype(np.float32)
    pad = KSIZE // 2
    Bm = np.zeros((GRID, GRID), dtype=np.float32)
    for i in range(GRID):
        for t in range(-pad, pad + 1):
            j = i + t
            if j < 0:
                j = -j
            elif j > GRID - 1:
                j = 2 * (GRID - 1) - j
            Bm[i, j] += g[t + pad]
    return Bm


def _build():
    import concourse.bacc as bacc
    import concourse.mybir as mybir
    import concourse.tile as tile

    f32 = mybir.dt.float32
    f32r = mybir.dt.float32r
    AF = mybir.ActivationFunctionType

    nc = bacc.Bacc("TRN2", target_bir_lowering=False, debug=False)

    # float32r end-to-end: same bits as fp32 (PE rounds internally), but it
    # satisfies the BIR verifier's "rounded to FP32r" producer rule and runs
    # the PE at full rate (1 cyc/row at N>=256) instead of fp32's 4 cyc/row.
    xT = nc.dram_tensor("xT", [E, S], f32r, kind="ExternalInput")    # x[b].T
    wkT = nc.dram_tensor("wkT", [E, E], f32r, kind="ExternalInput")  # W_k.T
    wqT = nc.dram_tensor("wqT", [E, E], f32r, kind="ExternalInput")  # W_q.T
    # blur-matrix generators: ab[s', 0:32] = Bm.T[s'//32, :], ab[s', 32:64] =
    # Bm.T[s'%32, :]; M^T[s', 32r+c] = ab[s', r] * ab[s', 32+c] is built on
    # device, saving a 4 MiB DMA on the startup critical path.
    ab = nc.dram_tensor("ab", [S, 2 * GRID], f32, kind="ExternalInput")
    out = nc.dram_tensor("out", [H, S, S], f32, kind="ExternalOutput")

    with tile.TileContext(nc) as tc:
        with (
            tc.tile_pool(name="persist", bufs=1) as pp,
            tc.tile_pool(name="qkb", bufs=2) as qp,
            tc.tile_pool(name="work", bufs=10) as wp,
            tc.tile_pool(name="stat", bufs=12) as sp,
            tc.tile_pool(name="pa", bufs=2, space="PSUM") as pa,
            tc.tile_pool(name="plg", bufs=3, space="PSUM") as plg,
        ):
            # ---- stage inputs in SBUF (ab first: tiny, unblocks the blur-
            # matrix build; x+wk next: the k-phase critical path; wq last)
            gts = []
            for i in range(NS):
                g = pp.tile([128, 2 * GRID], f32, tag=f"g{i}", name=f"g{i}")
                nc.sync.dma_start(g[:], ab[i * 128:(i + 1) * 128, :])
                gts.append(g)
            xts = []
            for i in range(NE):
                t = pp.tile([128, S], f32r, tag=f"x{i}", name=f"x{i}")
                nc.sync.dma_start(t[:], xT[i * 128:(i + 1) * 128, :])
                xts.append(t)
            # wk cols 0:256 arrive first (all the first k-chunk needs); the
            # rest of wk comes after wq so the first outputs start sooner
            wkts = []
            for i in range(NE):
                t = pp.tile([128, E], f32r, tag=f"wk{i}", name=f"wk{i}")
                nc.sync.dma_start(t[:, 0:256], wkT[i * 128:(i + 1) * 128, 0:256])
                wkts.append(t)
            wqts = []
            for i in range(NE):
                t = pp.tile([128, E], f32r, tag=f"wq{i}", name=f"wq{i}")
                nc.sync.dma_start(t[:], wqT[i * 128:(i + 1) * 128, :])
                wqts.append(t)
            for i in range(NE):
                nc.sync.dma_start(
                    wkts[i][:, 256:E], wkT[i * 128:(i + 1) * 128, 256:E]
                )

            # ---- build the blur matrix M^T in SBUF from the tiny generators
            mts = []
            for i in range(NS):
                g = gts[i]
                t = pp.tile([128, S], f32r, tag=f"m{i}", name=f"m{i}")
                nc.vector.tensor_mul(
                    t[:].rearrange("p (r c) -> p r c", c=GRID),
                    g[:, 0:GRID].to_broadcast([128, GRID, GRID]),
                    g[:].rearrange("p (a c) -> p a c", a=2)[:, 1:2, :]
                        .to_broadcast([128, GRID, GRID]),
                )
                mts.append(t)

            kts = []
            for st in range(NS):
                t = pp.tile([128, E], f32r, tag=f"k{st}", name=f"k{st}")
                kts.append(t)

            def k_st(f0, fn, st):
                # one s-tile's worth of k[:, f0:f0+fn] (6 matmuls + copy)
                ps = pa.tile([128, 512], f32, tag="pa", name=f"psk{st}_{f0}")
                for et in range(NE):
                    nc.tensor.matmul(
                        ps[:, 0:fn],
                        xts[et][:, st * 128:(st + 1) * 128],
                        wkts[et][:, f0:f0 + fn],
                        start=(et == 0),
                        stop=(et == NE - 1),
                    )
                if st % 2 == 0:
                    nc.scalar.copy(kts[st][:, f0:f0 + fn], ps[:, 0:fn])
                else:
                    nc.vector.tensor_copy(kts[st][:, f0:f0 + fn], ps[:, 0:fn])

            def k_block(f0, fn):
                for st in range(NS):
                    k_st(f0, fn, st)

            def q_tile(ft):
                return qp.tile([128, S], f32r, tag="qT", name=f"qT{ft}", bufs=5)

            def q_sb(qt, ft, sb):
                # one 512-wide block of qT[ft] (6 matmuls + copy)
                ps = pa.tile([128, 512], f32, tag="pa", name=f"psq{ft}_{sb}")
                for et in range(NE):
                    nc.tensor.matmul(
                        ps[:],
                        wqts[et][:, ft * 128:(ft + 1) * 128],
                        xts[et][:, sb * 512:(sb + 1) * 512],
                        start=(et == 0),
                        stop=(et == NE - 1),
                    )
                nc.vector.tensor_copy(qt[:, sb * 512:(sb + 1) * 512], ps[:])

            def q_proj(ft):
                qt = q_tile(ft)
                for sb in range(2):
                    q_sb(qt, ft, sb)
                return qt

            def head_pair(ft, qt, fillers=()):
                # blur: kbT[ft] = ((Bm x Bm) @ k)^T rows [128ft, 128ft+128)
                kbt = qp.tile([128, S], f32r, tag="kbT", name=f"kbT{ft}")
                for sb in range(2):
                    ps = pa.tile([128, 512], f32, tag="pa", name=f"psb{ft}_{sb}")
                    for st in range(NS):
                        nc.tensor.matmul(
                            ps[:],
                            kts[st][:, ft * 128:(ft + 1) * 128],
                            mts[st][:, sb * 512:(sb + 1) * 512],
                            start=(st == 0),
                            stop=(st == NS - 1),
                        )
                    nc.vector.tensor_copy(kbt[:, sb * 512:(sb + 1) * 512], ps[:])

                # two heads interleaved: adjacent logits matmuls sit in
                # different PE row groups (d-offset 0 vs 64). Filler work
                # (later k columns / qT projections) is sprinkled between
                # query chunks to keep PE density uniform against the
                # output-DMA drain rate.
                emitted = 0
                for qc in range(NS):
                    target = (qc + 1) * len(fillers) // NS
                    while emitted < target:
                        fillers[emitted]()
                        emitted += 1
                    for hh in range(2):
                        h = 2 * ft + hh
                        off = hh * D
                        lg = plg.tile([128, S], f32, tag="lg", name=f"lg{h}_{qc}")
                        for kb in range(2):
                            nc.tensor.matmul(
                                lg[:, kb * 512:(kb + 1) * 512],
                                qt[off:off + D, qc * 128:(qc + 1) * 128],
                                kbt[off:off + D, kb * 512:(kb + 1) * 512],
                                start=True,
                                stop=True,
                            )
                        ex = wp.tile([128, S], f32, tag="exp", name=f"ex{h}_{qc}")
                        acc = sp.tile([128, 1], f32, tag="acc", name=f"ac{h}_{qc}")
                        nc.scalar.activation(
                            ex[:], lg[:], AF.Exp, scale=SCALE, accum_out=acc[:]
                        )
                        rs = sp.tile([128, 1], f32, tag="rs", name=f"rs{h}_{qc}")
                        nc.vector.reciprocal(rs[:], acc[:])
                        nc.vector.tensor_scalar_mul(ex[:], ex[:], rs[:])
                        nc.sync.dma_start(out[h, qc * 128:(qc + 1) * 128, :], ex[:])

            # minimal chain to the first output: k cols 0:256 -> ft0 -> ft1.
            # All remaining k columns and qT projections are emitted as
            # fillers inside the head-pair loops, one ~6-matmul group per
            # query chunk, so the PE workload is uniform and the output DMA
            # never starves.
            k_block(0, 256)
            qt0 = q_proj(0)
            qt1 = q_proj(1)
            qt2, qt3, qt4, qt5 = (q_tile(ft) for ft in range(2, NF))
            f1 = [lambda st=st: k_st(256, 256, st) for st in range(NS)]
            f1 += [lambda sb=sb: q_sb(qt2, 2, sb) for sb in range(2)]
            f2 = [lambda sb=sb: q_sb(qt3, 3, sb) for sb in range(2)]
            f2 += [lambda st=st: k_st(512, 256, st) for st in range(NS)]
            f3 = [lambda sb=sb: q_sb(qt4, 4, sb) for sb in range(2)]
            f3 += [lambda sb=sb: q_sb(qt5, 5, sb) for sb in range(2)]
            head_pair(0, qt0)
            head_pair(1, qt1, f1)
            head_pair(2, qt2, f2)
            head_pair(3, qt3, f3)
            head_pair(4, qt4)
            head_pair(5, qt5)

    nc.compile()
    return nc


_CACHE = {}


def _get_nc():
    if "nc" not in _CACHE:
        _CACHE["nc"] = _build()
    return _CACHE["nc"]


def _make_in_maps(x, W_in):
    x = np.ascontiguousarray(np.asarray(x), dtype=np.float32)
    W_in = np.ascontiguousarray(np.asarray(W_in), dtype=np.float32)
    Bm = _blur_matrix_1d()
    # M^T[s', 32r+c] = Bm[r, s'//32] * Bm[c, s'%32]; ship the two 32-wide
    # generator tables and take the outer product on device.
    sp = np.arange(S)
    abn = np.concatenate([Bm.T[sp // GRID, :], Bm.T[sp % GRID, :]], axis=1)
    abn = np.ascontiguousarray(abn, dtype=np.float32)     # [S, 64]
    wkTn = np.ascontiguousarray(W_in[E:2 * E, :].T)       # [E, E]
    wqTn = np.ascontiguousarray(W_in[2 * E:3 * E, :].T)   # [E, E]
    in_maps = []
    for b in range(N_CORES):
        in_maps.append(
            {
                "xT": np.ascontiguousarray(x[b].T),
                "wkT": wkTn,
                "wqT": wqTn,
                "ab": abn,
            }
        )
    return in_maps


def _run(x, W_in, trace=False):
    from concourse.bass_utils import run_bass_kernel_spmd

    nc = _get_nc()
    in_maps = _make_in_maps(x, W_in)
    res = run_bass_kernel_spmd(nc, in_maps, list(range(N_CORES)), trace=trace)
    outs = [np.asarray(res.results[c]["out"]) for c in range(N_CORES)]
    full = np.concatenate(outs, axis=0)  # [B*H, S, S]
    return full, res


def kernel(x, W_in):
    full, _ = _run(x, W_in, trace=False)
    return full


# revision 20
# speedup vs baseline: 1.2407x; 1.0368x over previous
"""Trainium2 Bass kernel for MultiHeadedAttentionBlur.

Math: qkv = x @ W_in^T (chunks v,k,q); per-head logits = SCALE * q @ k^T;
logits' key axis viewed as a 32x32 grid gets a 5x5 reflect-padded gaussian
blur; softmax over keys.

Key identity used here: the blur is linear over the key axis, so
blur(q @ k^T) = q @ (M @ k)^T with M = Bm (x) Bm the 1024x1024 blur matrix
(kron of two 32x32 1-D reflect-blur matrices). Blurring k (S x 64 per head)
instead of logits (S x S per head) removes ~16x of the blur FLOPs, and the
M-matmul doubles as the transpose that the logits matmul needs.

Sharding: data-parallel over batch (B=8 -> 8 cores). Each core computes its
batch element's 12 heads: [12, 1024, 1024] slice of the full [96, 1024, 1024]
output.

Per-core device pipeline (all matmuls in float32r = full-rate fp32):
  1. k  = x @ Wk^T            -> [s, f] layout (s on partitions)
  2. qT = Wq @ x^T            -> [f, s] layout (f on partitions)
  3. kbT = (M @ k)^T via matmul(lhsT=k, rhs=M^T) -> [f, s] layout
  4. per head h, per 128-query chunk: logits = qT_h^T @ kbT_h (K=64),
     exp via ScalarE (scale=0.125 folded in, accum_out = row sums),
     normalize via VectorE reciprocal + tensor_scalar_mul, DMA out.
"""

import numpy as np

S = 1024
E = 768
H = 12
D = 64
B = 8
NE = E // 128  # 6 e-tiles (contraction tiles of the projection)
NS = S // 128  # 8 s-tiles
NF = E // 128  # 6 f-tiles (output feature tiles; 2 heads per tile)
SCALE = 0.125
N_CORES = 8
KSIZE = 5
SIGMA = 1.0
GRID = 32


def _blur_matrix_1d():
    # Matches reference gaussian_kernel1d (fp32) + 'reflect' padding.
    x = (np.arange(KSIZE, dtype=np.float32) - (KSIZE - 1) / 2.0).astype(np.float32)
    g = np.exp(-0.5 * (x / SIGMA) ** 2).astype(np.float32)
    g = (g / g.sum()).astype(np.float32)
    pad = KSIZE // 2
    Bm = np.zeros((GRID, GRID), dtype=np.float32)
    for i in range(GRID):
        for t in range(-pad, pad + 1):
            j = i + t
            if j < 0:
                j = -j
            elif j > GRID - 1:
                j = 2 * (GRID - 1) - j
            Bm[i, j] += g[t + pad]
    return Bm


def _build():
    import concourse.bacc as bacc
    import concourse.mybir as mybir
    import concourse.tile as tile

    f32 = mybir.dt.float32
    f32r = mybir.dt.float32r
    AF = mybir.ActivationFunctionType

    nc = bacc.Bacc("TRN2", target_bir_lowering=False, debug=False)

    # float32r end-to-end: same bits as fp32 (PE rounds internally), but it
    # satisfies the BIR verifier's "rounded to FP32r" producer rule and runs
    # the PE at full rate (1 cyc/row at N>=256) instead of fp32's 4 cyc/row.
    xT = nc.dram_tensor("xT", [E, S], f32r, kind="ExternalInput")    # x[b].T
    wkT = nc.dram_tensor("wkT", [E, E], f32r, kind="ExternalInput")  # W_k.T
    wqT = nc.dram_tensor("wqT", [E, E], f32r, kind="ExternalInput")  # W_q.T
    # blur-matrix generators: ab[s', 0:32] = Bm.T[s'//32, :], ab[s', 32:64] =
    # Bm.T[s'%32, :]; M^T[s', 32r+c] = ab[s', r] * ab[s', 32+c] is built on
    # device, saving a 4 MiB DMA on the startup critical path.
    ab = nc.dram_tensor("ab", [S, 2 * GRID], f32, kind="ExternalInput")
    out = nc.dram_tensor("out", [H, S, S], f32, kind="ExternalOutput")

    with tile.TileContext(nc) as tc:
        with (
            tc.tile_pool(name="persist", bufs=1) as pp,
            tc.tile_pool(name="qkb", bufs=2) as qp,
            tc.tile_pool(name="work", bufs=10) as wp,
            tc.tile_pool(name="stat", bufs=12) as sp,
            tc.tile_pool(name="pa", bufs=2, space="PSUM") as pa,
            tc.tile_pool(name="plg", bufs=3, space="PSUM") as plg,
        ):
            # ---- stage inputs in SBUF (ab first: tiny, unblocks the blur-
            # matrix build; x+wk next: the k-phase critical path; wq last)
            gts = []
            for i in range(NS):
                g = pp.tile([128, 2 * GRID], f32, tag=f"g{i}", name=f"g{i}")
                nc.sync.dma_start(g[:], ab[i * 128:(i + 1) * 128, :])
                gts.append(g)
            xts = []
            for i in range(NE):
                t = pp.tile([128, S], f32r, tag=f"x{i}", name=f"x{i}")
                nc.sync.dma_start(t[:], xT[i * 128:(i + 1) * 128, :])
                xts.append(t)
            # wk cols 0:256 arrive first (all the first k-chunk needs); the
            # rest of wk comes after wq so the first outputs start sooner
            wkts = []
            for i in range(NE):
                t = pp.tile([128, E], f32r, tag=f"wk{i}", name=f"wk{i}")
                nc.sync.dma_start(t[:, 0:256], wkT[i * 128:(i + 1) * 128, 0:256])
                wkts.append(t)
            wqts = []
            for i in range(NE):
                t = pp.tile([128, E], f32r, tag=f"wq{i}", name=f"wq{i}")
                nc.sync.dma_start(t[:], wqT[i * 128:(i + 1) * 128, :])
                wqts.append(t)
            for i in range(NE):
                nc.sync.dma_start(
                    wkts[i][:, 256:E], wkT[i * 128:(i + 1) * 128, 256:E]
                )

            # ---- build the blur matrix M^T in SBUF from the tiny generators
            mts = []
            for i in range(NS):
                g = gts[i]
                t = pp.tile([128, S], f32r, tag=f"m{i}", name=f"m{i}")
                nc.vector.tensor_mul(
                    t[:].rearrange("p (r c) -> p r c", c=GRID),
                    g[:, 0:GRID].to_broadcast([128, GRID, GRID]),
                    g[:].rearrange("p (a c) -> p a c", a=2)[:, 1:2, :]
                        .to_broadcast([128, GRID, GRID]),
                )
                mts.append(t)

            kts = []
            for st in range(NS):
                t = pp.tile([128, E], f32r, tag=f"k{st}", name=f"k{st}")
                kts.append(t)

            def k_st(f0, fn, st):
                # one s-tile's worth of k[:, f0:f0+fn] (6 matmuls + copy)
                ps = pa.tile([128, 512], f32, tag="pa", name=f"psk{st}_{f0}")
                for et in range(NE):
                    nc.tensor.matmul(
                        ps[:, 0:fn],
                        xts[et][:, st * 128:(st + 1) * 128],
                        wkts[et][:, f0:f0 + fn],
                        start=(et == 0),
                        stop=(et == NE - 1),
                    )
                if st % 2 == 0:
                    nc.scalar.copy(kts[st][:, f0:f0 + fn], ps[:, 0:fn])
                else:
                    nc.vector.tensor_copy(kts[st][:, f0:f0 + fn], ps[:, 0:fn])

            def k_block(f0, fn):
                for st in range(NS):
                    k_st(f0, fn, st)

            def q_tile(ft):
                return qp.tile([128, S], f32r, tag="qT", name=f"qT{ft}", bufs=5)

            def q_sb(qt, ft, sb):
                # one 512-wide block of qT[ft] (6 matmuls + copy)
                ps = pa.tile([128, 512], f32, tag="pa", name=f"psq{ft}_{sb}")
                for et in range(NE):
                    nc.tensor.matmul(
                        ps[:],
                        wqts[et][:, ft * 128:(ft + 1) * 128],
                        xts[et][:, sb * 512:(sb + 1) * 512],
                        start=(et == 0),
                        stop=(et == NE - 1),
                    )
                nc.vector.tensor_copy(qt[:, sb * 512:(sb + 1) * 512], ps[:])

            def q_proj(ft):
                qt = q_tile(ft)
                for sb in range(2):
                    q_sb(qt, ft, sb)
                return qt

            def head_pair(ft, qt, fillers=()):
                # blur: kbT[ft] = ((Bm x Bm) @ k)^T rows [128ft, 128ft+128).
                # Bm is banded (+-2 grid rows, reflect stays in-band), so an
                # output block sb only receives from s'-tiles whose grid rows
                # overlap [16*sb - 2, 16*sb + 18): st 0..4 for sb=0, st 3..7
                # for sb=1. The other tiles multiply exact zeros - skip them.
                kbt = qp.tile([128, S], f32r, tag="kbT", name=f"kbT{ft}")
                for sb in range(2):
                    sts = range(0, 5) if sb == 0 else range(3, NS)
                    ps = pa.tile([128, 512], f32, tag="pa", name=f"psb{ft}_{sb}")
                    for j, st in enumerate(sts):
                        nc.tensor.matmul(
                            ps[:],
                            kts[st][:, ft * 128:(ft + 1) * 128],
                            mts[st][:, sb * 512:(sb + 1) * 512],
                            start=(j == 0),
                            stop=(j == len(sts) - 1),
                        )
                    nc.vector.tensor_copy(kbt[:, sb * 512:(sb + 1) * 512], ps[:])

                # two heads interleaved: adjacent logits matmuls sit in
                # different PE row groups (d-offset 0 vs 64). Filler work
                # (later k columns / qT projections) is sprinkled between
                # query chunks to keep PE density uniform against the
                # output-DMA drain rate.
                emitted = 0
                for qc in range(NS):
                    target = (qc + 1) * len(fillers) // NS
                    while emitted < target:
                        fillers[emitted]()
                        emitted += 1
                    for hh in range(2):
                        h = 2 * ft + hh
                        off = hh * D
                        lg = plg.tile([128, S], f32, tag="lg", name=f"lg{h}_{qc}")
                        for kb in range(2):
                            nc.tensor.matmul(
                                lg[:, kb * 512:(kb + 1) * 512],
                                qt[off:off + D, qc * 128:(qc + 1) * 128],
                                kbt[off:off + D, kb * 512:(kb + 1) * 512],
                                start=True,
                                stop=True,
                            )
                        ex = wp.tile([128, S], f32, tag="exp", name=f"ex{h}_{qc}")
                        acc = sp.tile([128, 1], f32, tag="acc", name=f"ac{h}_{qc}")
                        nc.scalar.activation(
                            ex[:], lg[:], AF.Exp, scale=SCALE, accum_out=acc[:]
                        )
                        rs = sp.tile([128, 1], f32, tag="rs", name=f"rs{h}_{qc}")
                        nc.vector.reciprocal(rs[:], acc[:])
                        nc.vector.tensor_scalar_mul(ex[:], ex[:], rs[:])
                        nc.sync.dma_start(out[h, qc * 128:(qc + 1) * 128, :], ex[:])

            # warm the ACT exp table while inputs stream in
            scratch = sp.tile([128, 1], f32, tag="warm", name="warmup")
            nc.scalar.activation(scratch[:], gts[0][:, 0:1], AF.Exp)

            # minimal chain to the first output: k cols 0:256 -> ft0 -> ft1.
            # The remaining k columns (512-wide, cheaper per flop) and qT
            # projections are emitted as fillers inside the head-pair loops,
            # one ~6-matmul group per query chunk, so the PE workload stays
            # uniform against the output-DMA drain rate.
            k_block(0, 256)
            qt0 = q_proj(0)
            qt1 = q_proj(1)
            qt2, qt3, qt4, qt5 = (q_tile(ft) for ft in range(2, NF))
            f0 = [lambda st=st: k_st(256, 512, st) for st in range(5)]
            f1 = [lambda st=st: k_st(256, 512, st) for st in range(5, NS)]
            f1 += [lambda sb=sb: q_sb(qt2, 2, sb) for sb in range(2)]
            f2 = [lambda sb=sb: q_sb(qt3, 3, sb) for sb in range(2)]
            f2 += [lambda sb=sb: q_sb(qt4, 4, sb) for sb in range(2)]
            f3 = [lambda sb=sb: q_sb(qt5, 5, sb) for sb in range(2)]
            head_pair(0, qt0, f0)
            head_pair(1, qt1, f1)
            head_pair(2, qt2, f2)
            head_pair(3, qt3, f3)
            head_pair(4, qt4)
            head_pair(5, qt5)

    nc.compile()
    return nc


_CACHE = {}


def _get_nc():
    if "nc" not in _CACHE:
        _CACHE["nc"] = _build()
    return _CACHE["nc"]


def _make_in_maps(x, W_in):
    x = np.ascontiguousarray(np.asarray(x), dtype=np.float32)
    W_in = np.ascontiguousarray(np.asarray(W_in), dtype=np.float32)
    Bm = _blur_matrix_1d()
    # M^T[s', 32r+c] = Bm[r, s'//32] * Bm[c, s'%32]; ship the two 32-wide
    # generator tables and take the outer product on device.
    sp = np.arange(S)
    abn = np.concatenate([Bm.T[sp // GRID, :], Bm.T[sp % GRID, :]], axis=1)
    abn = np.ascontiguousarray(abn, dtype=np.float32)     # [S, 64]
    wkTn = np.ascontiguousarray(W_in[E:2 * E, :].T)       # [E, E]
    wqTn = np.ascontiguousarray(W_in[2 * E:3 * E, :].T)   # [E, E]
    in_maps = []
    for b in range(N_CORES):
        in_maps.append(
            {
                "xT": np.ascontiguousarray(x[b].T),
                "wkT": wkTn,
                "wqT": wqTn,
                "ab": abn,
            }
        )
    return in_maps


def _run(x, W_in, trace=False):
    from concourse.bass_utils import run_bass_kernel_spmd

    nc = _get_nc()
    in_maps = _make_in_maps(x, W_in)
    res = run_bass_kernel_spmd(nc, in_maps, list(range(N_CORES)), trace=trace)
    outs = [np.asarray(res.results[c]["out"]) for c in range(N_CORES)]
    full = np.concatenate(outs, axis=0)  # [B*H, S, S]
    return full, res


def kernel(x, W_in):
    full, _ = _run(x, W_in, trace=False)
    return full
